# revision 28
# baseline (speedup 1.0000x reference)
"""Trainium2 Bass kernel for nn_MultiHeadAttention (B=4,H=16,S=2048,PHD=64).

Strategy (8 cores, no collectives):
  - core c handles batch b=c//2 and a balanced half of the causal-triangle
    query blocks: parity p=c%2 picks q-tiles {2i+p} U {15-(2i+p)}, whose causal
    work sums equally for both parities.
  - scores^T = T2^T q^T with T2 = k @ (scale*Wk@Wq^T) precomputed on host
    (projections are O(S) work; attention O(S^2) stays on device).  Per-q bias
    terms drop out of softmax (shift invariance); the per-k bias k.(Wk bq)
    becomes the exp() bias.
  - Lazy softmax (no max subtraction -- logits are tiny): E = exp(s^T + w),
    PV via Vt^T @ E with an appended ones column producing row sums, then a
    per-head normalize, head-pair-packed row-parallel output projection with
    bo added on device.  All matmul inputs bf16, fp32 PSUM accumulation.
  - The SPMD program is identical across cores; parity differences are
    expressed purely through data ({0,1} mask tiles for the last two
    key-blocks of every q-tile position).  q-tile positions are sorted, so at
    key-block j the valid positions form a suffix [j//2, 8) and PV runs as
    bank-wide matmuls over that suffix -- causality needs no per-tile splits.
  - Host does layout transforms / small projections and gathers the disjoint
    output rows.  Falls back to a mask-from-data full-block program if the
    mask input is not exactly causal.
"""

import numpy as np
import sys

for _p in ("/opt/trn_rl_repo", "/root/.axon_site/_ro/trn_rl_repo"):
    if _p not in sys.path:
        sys.path.insert(0, _p)

import ml_dtypes

import concourse.bass as bass
import concourse.bacc as bacc
import concourse.mybir as mybir
import concourse.tile as tile
from concourse.bass_utils import run_bass_kernel_spmd

BF = ml_dtypes.bfloat16
B, H, S, PHD = 4, 16, 2048, 64
QK_IN = 2 * PHD          # 128
DM = H * PHD             # 1024
SCALE = np.float32(1.0 / np.sqrt(np.float32(QK_IN)))
NT = S // 128            # 16 key blocks
NPOS = 8                 # q-tile positions per core
NQ = NPOS * 128          # 1024 query rows per core
NCORES = 8


def _core_tiles(parity: int) -> list[int]:
    return sorted([2 * i + parity for i in range(4)] + [15 - (2 * i + parity) for i in range(4)])


def _chunks_from(c0):
    """Bank-aligned (start, size) chunks covering [c0, NQ) with 512 boundaries."""
    out = []
    pos = c0
    while pos < NQ:
        end = min((pos // 512 + 1) * 512, NQ)
        out.append((pos, end - pos))
        pos = end
    return out


def _build_program(blocks_per_pos, masked, nmask):
    """blocks_per_pos[i]: #key-blocks for position i (positions sorted by it).
    masked[(i, j)] -> mask slot for position-i block-j."""
    f32, bf16 = mybir.dt.float32, mybir.dt.bfloat16
    nc = bacc.Bacc("TRN2", target_bir_lowering=False, debug=False)

    # first valid position at block j (suffix property must hold)
    def imin(j):
        v = [i for i in range(NPOS) if blocks_per_pos[i] > j]
        return min(v) if v else None

    qT_d = nc.dram_tensor("qT", [H, 128, NQ], bf16, kind="ExternalInput").ap()
    T2_d = nc.dram_tensor("T2T", [H, 128, S], bf16, kind="ExternalInput").ap()
    Vt_d = nc.dram_tensor("Vt", [H, 128, NT * 65], bf16, kind="ExternalInput").ap()
    mk_d = nc.dram_tensor("mk", [128, max(nmask, 1) * 128], bf16, kind="ExternalInput").ap()
    Wo_d = nc.dram_tensor("WoT", [8, 128, DM], bf16, kind="ExternalInput").ap()
    bo_d = nc.dram_tensor("bo", [1, DM], f32, kind="ExternalInput").ap()
    out_d = nc.dram_tensor("out", [NPOS, 128, DM], f32, kind="ExternalOutput").ap()

    with tile.TileContext(nc) as tc:
        with (
            tc.tile_pool(name="const", bufs=1) as constp,
            tc.tile_pool(name="stack", bufs=1) as stackp,
            tc.tile_pool(name="perhead", bufs=3) as headp,
            tc.tile_pool(name="esb", bufs=8) as ep,
            tc.tile_pool(name="outsb", bufs=4) as outp,
            tc.tile_pool(name="rsb", bufs=2) as rsp,
            tc.tile_pool(name="rsd", bufs=2, space="DRAM") as rsdp,
            tc.tile_pool(name="ps", bufs=2, space="PSUM") as psp,
            tc.tile_pool(name="pso", bufs=1, space="PSUM") as psop,
            tc.tile_pool(name="pop", bufs=2, space="PSUM") as pop,
        ):
            # ---- head-0 loads first (shortest path to compute), then consts
            def _head_loads(h):
                # spread across HWDGE (sync) and SWDGE (gpsimd) queues so the
                # three loads stream in parallel
                T2T = headp.tile([128, S], bf16, tag="T2T", name=f"T2T{h}")
                nc.sync.dma_start(out=T2T, in_=T2_d[h])
                qT_sb = headp.tile([128, NQ], bf16, tag="qT", name=f"qT{h}")
                nc.gpsimd.dma_start(out=qT_sb, in_=qT_d[h])
                Vt = headp.tile([128, NT, 65], bf16, tag="Vt", name=f"Vt{h}")
                nc.gpsimd.dma_start(out=Vt, in_=Vt_d[h])
                return T2T, qT_sb, Vt

            h0_tiles = _head_loads(0)
            mk_sb = constp.tile([128, max(nmask, 1) * 128], bf16)
            nc.sync.dma_start(out=mk_sb, in_=mk_d)
            oT_stack = [stackp.tile([128, NQ], bf16, tag=f"ot{pair}", name=f"ot{pair}")
                        for pair in range(8)]
            WoT_sb = [None] * 8
            bo_sb = None
            acc_sb = {}

            def _outproj_unit(t, ch, first):
                prs = range(0, 4) if first else range(4, 8)
                po = pop.tile([128, 512], f32, tag="po", name=f"po{t}_{ch}_{int(first)}")
                for pi, pair in enumerate(prs):
                    nc.tensor.matmul(po, oT_stack[pair][:, t * 128:(t + 1) * 128],
                                     WoT_sb[pair][:, ch * 512:(ch + 1) * 512],
                                     start=(pi == 0), stop=(pi == 3))
                if first:
                    acc = outp.tile([128, 512], f32, tag="acc", name=f"acc{t}_{ch}", bufs=16)
                    nc.vector.tensor_add(acc, po, bo_sb[:, ch * 512:(ch + 1) * 512])
                    acc_sb[(t, ch)] = acc
                else:
                    ot = outp.tile([128, 512], f32, tag="osb", name=f"osb{t}_{ch}")
                    nc.vector.tensor_add(ot, po, acc_sb[(t, ch)])
                    nc.gpsimd.dma_start(out=out_d[t, :, ch * 512:(ch + 1) * 512], in_=ot)

            # ---- per-head attention ----
            for h in range(H):
                T2T, qT_sb, Vt = h0_tiles if h == 0 else _head_loads(h)

                # Attention over key blocks.  exp(w) is folded into Vt on the
                # host, so exp needs no bias; for the late key blocks (cols
                # <= 512) two blocks' scores pack into one 2-bank psum tile
                # and share a single exp op (the 185ns/op PSUM-access cost is
                # significant).  PV for one group is emitted after the next
                # group's scores/exp so PE never waits on ACT.
                oT = psop.tile([65, NQ], f32, tag="oT")
                pending = []  # deferred PV units: (E, j, e_off, c0, cols)
                def _pv_flush():
                    for Epv, pj, e_off, pc0, pcols in pending:
                        for pos, csz in _chunks_from(pc0):
                            if pos >= pc0 + pcols:
                                break
                            nc.tensor.matmul(
                                oT[:, pos:pos + csz],
                                Vt[:, pj, :],
                                Epv[:, e_off + (pos - pc0):e_off + (pos - pc0) + csz],
                                start=(pj == 0), stop=(pj == NT - 1),
                                skip_group_check=True)
                    pending.clear()
                def _masks(E, j, e_off, c0):
                    i0 = c0 // 128
                    for i in range(i0, NPOS):
                        if (i, j) in masked:
                            slot = masked[(i, j)]
                            sl = slice(e_off + (i - i0) * 128, e_off + (i - i0 + 1) * 128)
                            nc.vector.tensor_mul(E[:, sl], E[:, sl],
                                                 mk_sb[:, slot * 128:(slot + 1) * 128])
                quad_done = False
                for m in range(NT // 2):
                    j0, j1 = 2 * m, 2 * m + 1
                    if m == 7 and quad_done:
                        continue
                    if m == 6 and imin(12) == 6 and imin(14) == 7:
                        # quad-pack j12..15 (256+256+128+128 cols) into one
                        # tile with a single contiguous exp op
                        quad_done = True
                        ps = psp.tile([128, NQ], f32, tag="ps")
                        E = ep.tile([128, NQ], bf16, tag="E")
                        offs = [(12, 0, 768, 256), (13, 256, 768, 256),
                                (14, 512, 896, 128), (15, 640, 896, 128)]
                        for (jq, e_off, qc0, qw) in offs:
                            nc.tensor.matmul(ps[:, e_off:e_off + qw],
                                             T2T[:, jq * 128:(jq + 1) * 128],
                                             qT_sb[:, qc0:qc0 + qw], start=True, stop=True)
                        nc.scalar.activation(out=E[:, 0:768], in_=ps[:, 0:768],
                                             func=mybir.ActivationFunctionType.Exp)
                        _pv_flush()
                        for (jq, e_off, qc0, qw) in offs:
                            _masks(E, jq, e_off, qc0)
                            pending.append((E, jq, e_off, qc0, qw))
                        continue
                    i0 = imin(j0)
                    if i0 is None:
                        continue
                    assert i0 == imin(j1) if imin(j1) is not None else True
                    c0 = i0 * 128
                    cols = NQ - c0
                    if cols <= 512:
                        # packed pair: halves at offsets 0 and 512
                        ps = psp.tile([128, NQ], f32, tag="ps")
                        nc.tensor.matmul(ps[:, 0:cols], T2T[:, j0 * 128:(j0 + 1) * 128],
                                         qT_sb[:, c0:], start=True, stop=True)
                        nc.tensor.matmul(ps[:, 512:512 + cols], T2T[:, j1 * 128:(j1 + 1) * 128],
                                         qT_sb[:, c0:], start=True, stop=True)
                        E = ep.tile([128, NQ], bf16, tag="E")
                        psv = ps.rearrange("p (two c) -> p two c", two=2)[:, :, 0:cols]
                        Ev = E.rearrange("p (two c) -> p two c", two=2)[:, :, 0:cols]
                        nc.scalar.activation(out=Ev, in_=psv,
                                             func=mybir.ActivationFunctionType.Exp)
                        _pv_flush()
                        _masks(E, j0, 0, c0)
                        _masks(E, j1, 512, c0)
                        pending.append((E, j0, 0, c0, cols))
                        pending.append((E, j1, 512, c0, cols))
                    else:
                        for j in (j0, j1):
                            ps = psp.tile([128, NQ], f32, tag="ps")
                            for pos, csz in _chunks_from(c0):
                                nc.tensor.matmul(ps[:, pos:pos + csz],
                                                 T2T[:, j * 128:(j + 1) * 128],
                                                 qT_sb[:, pos:pos + csz],
                                                 start=True, stop=True)
                            E = ep.tile([128, NQ], bf16, tag="E")
                            nc.scalar.activation(out=E[:, c0:], in_=ps[:, c0:],
                                                 func=mybir.ActivationFunctionType.Exp)
                            _pv_flush()
                            _masks(E, j, c0, c0)
                            pending.append((E, j, c0, c0, cols))
                _pv_flush()

                # normalize + pack into head-pair stack (bf16).  One DVE copy
                # releases the single oT PSUM slot immediately; the rest of
                # the chain reads the SBUF copy.
                oc = rsp.tile([65, NQ], f32, tag="oc", bufs=3)
                nc.vector.tensor_copy(oc, oT)
                rs1 = rsp.tile([1, NQ], f32, tag="rs1")
                nc.vector.reciprocal(out=rs1, in_=oc[64:65, :])
                rsd = rsdp.tile([1, NQ], f32, tag="rsd")
                nc.gpsimd.dma_start(out=rsd, in_=rs1)
                rsb = rsp.tile([64, NQ], f32, tag="rsb")
                nc.gpsimd.dma_start(out=rsb, in_=rsd.to_broadcast([64, NQ]))
                half = (h % 2) * 64
                nc.vector.tensor_mul(oT_stack[h // 2][half:half + 64, :], oc[0:64, :], rsb)

                if h == 3:
                    # constant loads, needed from h==8 on
                    bo_sb = constp.tile([128, DM], f32, name="bo_sb")
                    nc.sync.dma_start(out=bo_sb, in_=bo_d.to_broadcast([128, DM]))
                    for pair in range(8):
                        t_ = constp.tile([128, DM], bf16, tag=f"wot{pair}", name=f"wot{pair}")
                        nc.sync.dma_start(out=t_, in_=Wo_d[pair])
                        WoT_sb[pair] = t_
                if 8 <= h <= 15:
                    # phase-1 outproj (pairs 0-3, ready after head 7): two
                    # units per head, hidden under ACT-bound attention via the
                    # dedicated pop pool
                    for u in ((h - 8) * 2, (h - 8) * 2 + 1):
                        _outproj_unit(u // 2, u % 2, True)

            # ---- output projection phase 2 (pairs 4-7) ----
            for t in range(NPOS):
                for ch in range(DM // 512):
                    _outproj_unit(t, ch, False)

    nc.compile()
    return nc


_PROG_CACHE = {}


def _get_program(causal: bool):
    key = bool(causal)
    if key not in _PROG_CACHE:
        if causal:
            blocks_per_pos = [2 * i + 2 for i in range(NPOS)]
            masked = {}
            for i in range(NPOS):
                masked[(i, 2 * i)] = 2 * i
                masked[(i, 2 * i + 1)] = 2 * i + 1
            nmask = 2 * NPOS
        else:
            blocks_per_pos = [NT] * NPOS
            masked = {(i, j): i * NT + j for i in range(NPOS) for j in range(NT)}
            nmask = NPOS * NT
        _PROG_CACHE[key] = (_build_program(blocks_per_pos, masked, nmask), masked, nmask)
    return _PROG_CACHE[key]


def _prep_inputs(q, k, v, Wq, bq, Wk, bk, Wv, bv, Wo, bo, mask, masked, nmask):
    A = (np.einsum('hde,hfe->hdf', Wk, Wq) * SCALE).astype(np.float32)   # [H,128,128]
    u = (np.einsum('hde,he->hd', Wk, bq) * SCALE).astype(np.float32)     # [H,128]
    WoT_host = np.ascontiguousarray(Wo.T.reshape(8, 128, DM)).astype(BF)
    bo_host = np.ascontiguousarray(bo[None, :]).astype(np.float32)
    mvalid = (mask[0, 0] != 0)                                           # [S(q), S(k)]

    in_maps = []
    tiles_by_core = []
    for c in range(NCORES):
        b, parity = c // 2, c % 2
        tiles = _core_tiles(parity)
        tiles_by_core.append(tiles)
        rows = np.concatenate([np.arange(t * 128, (t + 1) * 128) for t in tiles])
        qT = np.ascontiguousarray(q[b][:, rows, :].transpose(0, 2, 1)).astype(BF)
        # T2T[h] = (k[b,h] @ A_h)^T
        T2T = np.einsum('hsd,hdf->hfs', k[b], A).astype(BF)              # [H,128,S]
        # Vt[h, k_local, j, :] = [V[h, j*128+k_local, :], 1] * exp(w[h, j*128+k_local])
        # (the per-key exp bias is folded into Vt; softmax normalization
        # divides it back out exactly where it matters)
        V = (np.einsum('hsd,hde->hse', v[b], Wv) + bv[:, None, :]).astype(np.float32)
        wbv = np.exp(np.einsum('hsd,hd->hs', k[b], u)).astype(np.float32)  # [H,S]
        Vt = np.concatenate([V.reshape(H, NT, 128, PHD).transpose(0, 2, 1, 3),
                             np.ones((H, 128, NT, 1), np.float32)], axis=3)
        Vt *= wbv.reshape(H, NT, 128).transpose(0, 2, 1)[:, :, :, None]
        Vt = np.ascontiguousarray(Vt.reshape(H, 128, NT * 65)).astype(BF)
        mk_host = np.zeros((128, max(nmask, 1) * 128), np.float32)
        for (i, j), slot in masked.items():
            t = tiles[i]
            sub = mvalid[t * 128:(t + 1) * 128, j * 128:(j + 1) * 128]   # [q,k]
            mk_host[:, slot * 128:(slot + 1) * 128] = sub.T.astype(np.float32)
        in_maps.append({
            "qT": qT, "T2T": T2T, "Vt": Vt, "mk": mk_host.astype(BF),
            "WoT": WoT_host, "bo": bo_host,
        })
    return in_maps, tiles_by_core


def _is_causal(mask):
    m = np.asarray(mask[0, 0])
    expect = np.tri(S, S, dtype=np.int64)
    return bool(np.array_equal((m != 0), (expect != 0)))


def kernel(q, k, v, Wq, bq, Wk, bk, Wv, bv, Wo, bo, mask):
    q, k, v = (np.asarray(x, np.float32) for x in (q, k, v))
    Wq, bq, Wk, bk = (np.asarray(x, np.float32) for x in (Wq, bq, Wk, bk))
    Wv, bv, Wo, bo = (np.asarray(x, np.float32) for x in (Wv, bv, Wo, bo))
    mask = np.asarray(mask)

    causal = _is_causal(mask)
    nc, masked, nmask = _get_program(causal)
    in_maps, tiles_by_core = _prep_inputs(q, k, v, Wq, bq, Wk, bk, Wv, bv, Wo, bo,
                                          mask, masked, nmask)
    res = run_bass_kernel_spmd(nc, in_maps, core_ids=list(range(NCORES)))
    out_full = np.empty((B, S, DM), np.float32)
    for c in range(NCORES):
        b = c // 2
        oc = res.results[c]["out"]                                       # [NPOS,128,DM]
        for i, t in enumerate(tiles_by_core[c]):
            out_full[b, t * 128:(t + 1) * 128, :] = oc[i]
    return out_full


# revision 29
# speedup vs baseline: 1.0027x; 1.0027x over previous
"""Trainium2 Bass kernel for nn_MultiHeadAttention (B=4,H=16,S=2048,PHD=64).

Strategy (8 cores, no collectives):
  - core c handles batch b=c//2 and a balanced half of the causal-triangle
    query blocks: parity p=c%2 picks q-tiles {2i+p} U {15-(2i+p)}, whose causal
    work sums equally for both parities.
  - scores^T = T2^T q^T with T2 = k @ (scale*Wk@Wq^T) precomputed on host
    (projections are O(S) work; attention O(S^2) stays on device).  Per-q bias
    terms drop out of softmax (shift invariance); the per-k bias k.(Wk bq)
    becomes the exp() bias.
  - Lazy softmax (no max subtraction -- logits are tiny): E = exp(s^T + w),
    PV via Vt^T @ E with an appended ones column producing row sums, then a
    per-head normalize, head-pair-packed row-parallel output projection with
    bo added on device.  All matmul inputs bf16, fp32 PSUM accumulation.
  - The SPMD program is identical across cores; parity differences are
    expressed purely through data ({0,1} mask tiles for the last two
    key-blocks of every q-tile position).  q-tile positions are sorted, so at
    key-block j the valid positions form a suffix [j//2, 8) and PV runs as
    bank-wide matmuls over that suffix -- causality needs no per-tile splits.
  - Host does layout transforms / small projections and gathers the disjoint
    output rows.  Falls back to a mask-from-data full-block program if the
    mask input is not exactly causal.
"""

import numpy as np
import sys

for _p in ("/opt/trn_rl_repo", "/root/.axon_site/_ro/trn_rl_repo"):
    if _p not in sys.path:
        sys.path.insert(0, _p)

import ml_dtypes

import concourse.bass as bass
import concourse.bacc as bacc
import concourse.mybir as mybir
import concourse.tile as tile
from concourse.bass_utils import run_bass_kernel_spmd

BF = ml_dtypes.bfloat16
B, H, S, PHD = 4, 16, 2048, 64
QK_IN = 2 * PHD          # 128
DM = H * PHD             # 1024
SCALE = np.float32(1.0 / np.sqrt(np.float32(QK_IN)))
NT = S // 128            # 16 key blocks
NPOS = 8                 # q-tile positions per core
NQ = NPOS * 128          # 1024 query rows per core
NCORES = 8


def _core_tiles(parity: int) -> list[int]:
    return sorted([2 * i + parity for i in range(4)] + [15 - (2 * i + parity) for i in range(4)])


def _chunks_from(c0):
    """Bank-aligned (start, size) chunks covering [c0, NQ) with 512 boundaries."""
    out = []
    pos = c0
    while pos < NQ:
        end = min((pos // 512 + 1) * 512, NQ)
        out.append((pos, end - pos))
        pos = end
    return out


def _build_program(blocks_per_pos, masked, nmask):
    """blocks_per_pos[i]: #key-blocks for position i (positions sorted by it).
    masked[(i, j)] -> mask slot for position-i block-j."""
    f32, bf16 = mybir.dt.float32, mybir.dt.bfloat16
    nc = bacc.Bacc("TRN2", target_bir_lowering=False, debug=False)

    # first valid position at block j (suffix property must hold)
    def imin(j):
        v = [i for i in range(NPOS) if blocks_per_pos[i] > j]
        return min(v) if v else None

    qT_d = nc.dram_tensor("qT", [H, 128, NQ], bf16, kind="ExternalInput").ap()
    T2_d = nc.dram_tensor("T2T", [H, 128, S], bf16, kind="ExternalInput").ap()
    Vt_d = nc.dram_tensor("Vt", [H, 128, NT * 65], bf16, kind="ExternalInput").ap()
    mk_d = nc.dram_tensor("mk", [128, max(nmask, 1) * 128], bf16, kind="ExternalInput").ap()
    Wo_d = nc.dram_tensor("WoT", [8, 128, DM], bf16, kind="ExternalInput").ap()
    bo_d = nc.dram_tensor("bo", [1, DM], f32, kind="ExternalInput").ap()
    out_d = nc.dram_tensor("out", [NPOS, 128, DM], f32, kind="ExternalOutput").ap()

    with tile.TileContext(nc) as tc:
        with (
            tc.tile_pool(name="const", bufs=1) as constp,
            tc.tile_pool(name="stack", bufs=1) as stackp,
            tc.tile_pool(name="perhead", bufs=3) as headp,
            tc.tile_pool(name="esb", bufs=8) as ep,
            tc.tile_pool(name="outsb", bufs=4) as outp,
            tc.tile_pool(name="rsb", bufs=2) as rsp,
            tc.tile_pool(name="rsd", bufs=2, space="DRAM") as rsdp,
            tc.tile_pool(name="ps", bufs=2, space="PSUM") as psp,
            tc.tile_pool(name="pso", bufs=2, space="PSUM") as psop,
        ):
            # ---- head-0 loads first (shortest path to compute), then consts
            def _head_loads(h):
                # spread across HWDGE (sync) and SWDGE (gpsimd) queues so the
                # three loads stream in parallel
                T2T = headp.tile([128, S], bf16, tag="T2T", name=f"T2T{h}")
                nc.sync.dma_start(out=T2T, in_=T2_d[h])
                qT_sb = headp.tile([128, NQ], bf16, tag="qT", name=f"qT{h}")
                nc.gpsimd.dma_start(out=qT_sb, in_=qT_d[h])
                Vt = headp.tile([128, NT, 65], bf16, tag="Vt", name=f"Vt{h}")
                nc.gpsimd.dma_start(out=Vt, in_=Vt_d[h])
                return T2T, qT_sb, Vt

            h0_tiles = _head_loads(0)
            mk_sb = constp.tile([128, max(nmask, 1) * 128], bf16)
            nc.sync.dma_start(out=mk_sb, in_=mk_d)
            oT_stack = [stackp.tile([128, NQ], bf16, tag=f"ot{pair}", name=f"ot{pair}")
                        for pair in range(8)]
            WoT_sb = [None] * 8
            bo_sb = None
            acc_sb = {}

            # ---- per-head attention ----
            for h in range(H):
                T2T, qT_sb, Vt = h0_tiles if h == 0 else _head_loads(h)

                # Attention over key blocks.  exp(w) is folded into Vt on the
                # host, so exp needs no bias; for the late key blocks (cols
                # <= 512) two blocks' scores pack into one 2-bank psum tile
                # and share a single exp op (the 185ns/op PSUM-access cost is
                # significant).  PV for one group is emitted after the next
                # group's scores/exp so PE never waits on ACT.
                oT = psop.tile([65, NQ], f32, tag="oT")
                pending = []  # deferred PV units: (E, j, e_off, c0, cols)
                def _pv_flush():
                    for Epv, pj, e_off, pc0, pcols in pending:
                        for pos, csz in _chunks_from(pc0):
                            if pos >= pc0 + pcols:
                                break
                            nc.tensor.matmul(
                                oT[:, pos:pos + csz],
                                Vt[:, pj, :],
                                Epv[:, e_off + (pos - pc0):e_off + (pos - pc0) + csz],
                                start=(pj == 0), stop=(pj == NT - 1),
                                skip_group_check=True)
                    pending.clear()
                def _masks(E, j, e_off, c0):
                    i0 = c0 // 128
                    for i in range(i0, NPOS):
                        if (i, j) in masked:
                            slot = masked[(i, j)]
                            sl = slice(e_off + (i - i0) * 128, e_off + (i - i0 + 1) * 128)
                            nc.vector.tensor_mul(E[:, sl], E[:, sl],
                                                 mk_sb[:, slot * 128:(slot + 1) * 128])
                quad_done = False
                for m in range(NT // 2):
                    j0, j1 = 2 * m, 2 * m + 1
                    if m == 7 and quad_done:
                        continue
                    if m == 6 and imin(12) == 6 and imin(14) == 7:
                        # quad-pack j12..15 (256+256+128+128 cols) into one
                        # tile with a single contiguous exp op
                        quad_done = True
                        ps = psp.tile([128, NQ], f32, tag="ps")
                        E = ep.tile([128, NQ], bf16, tag="E")
                        offs = [(12, 0, 768, 256), (13, 256, 768, 256),
                                (14, 512, 896, 128), (15, 640, 896, 128)]
                        for (jq, e_off, qc0, qw) in offs:
                            nc.tensor.matmul(ps[:, e_off:e_off + qw],
                                             T2T[:, jq * 128:(jq + 1) * 128],
                                             qT_sb[:, qc0:qc0 + qw], start=True, stop=True)
                        nc.scalar.activation(out=E[:, 0:768], in_=ps[:, 0:768],
                                             func=mybir.ActivationFunctionType.Exp)
                        _pv_flush()
                        for (jq, e_off, qc0, qw) in offs:
                            _masks(E, jq, e_off, qc0)
                            pending.append((E, jq, e_off, qc0, qw))
                        continue
                    i0 = imin(j0)
                    if i0 is None:
                        continue
                    assert i0 == imin(j1) if imin(j1) is not None else True
                    c0 = i0 * 128
                    cols = NQ - c0
                    if cols <= 512:
                        # packed pair: halves at offsets 0 and 512
                        ps = psp.tile([128, NQ], f32, tag="ps")
                        nc.tensor.matmul(ps[:, 0:cols], T2T[:, j0 * 128:(j0 + 1) * 128],
                                         qT_sb[:, c0:], start=True, stop=True)
                        nc.tensor.matmul(ps[:, 512:512 + cols], T2T[:, j1 * 128:(j1 + 1) * 128],
                                         qT_sb[:, c0:], start=True, stop=True)
                        E = ep.tile([128, NQ], bf16, tag="E")
                        psv = ps.rearrange("p (two c) -> p two c", two=2)[:, :, 0:cols]
                        Ev = E.rearrange("p (two c) -> p two c", two=2)[:, :, 0:cols]
                        nc.scalar.activation(out=Ev, in_=psv,
                                             func=mybir.ActivationFunctionType.Exp)
                        _pv_flush()
                        _masks(E, j0, 0, c0)
                        _masks(E, j1, 512, c0)
                        pending.append((E, j0, 0, c0, cols))
                        pending.append((E, j1, 512, c0, cols))
                    else:
                        for j in (j0, j1):
                            ps = psp.tile([128, NQ], f32, tag="ps")
                            for pos, csz in _chunks_from(c0):
                                nc.tensor.matmul(ps[:, pos:pos + csz],
                                                 T2T[:, j * 128:(j + 1) * 128],
                                                 qT_sb[:, pos:pos + csz],
                                                 start=True, stop=True)
                            E = ep.tile([128, NQ], bf16, tag="E")
                            nc.scalar.activation(out=E[:, c0:], in_=ps[:, c0:],
                                                 func=mybir.ActivationFunctionType.Exp)
                            _pv_flush()
                            _masks(E, j, c0, c0)
                            pending.append((E, j, c0, c0, cols))
                _pv_flush()

                # normalize + pack into head-pair stack (bf16)
                rs1 = rsp.tile([1, NQ], f32, tag="rs1")
                nc.vector.reciprocal(out=rs1, in_=oT[64:65, :])
                rsd = rsdp.tile([1, NQ], f32, tag="rsd")
                nc.gpsimd.dma_start(out=rsd, in_=rs1)
                rsb = rsp.tile([64, NQ], f32, tag="rsb")
                nc.gpsimd.dma_start(out=rsb, in_=rsd.to_broadcast([64, NQ]))
                half = (h % 2) * 64
                nc.vector.tensor_mul(oT_stack[h // 2][half:half + 64, :], oT[0:64, :], rsb)

                if h == H - 3:
                    # late constant loads, needed only by the output projection
                    bo_sb = constp.tile([128, DM], f32, name="bo_sb")
                    nc.sync.dma_start(out=bo_sb, in_=bo_d.to_broadcast([128, DM]))
                    for pair in range(8):
                        t_ = constp.tile([128, DM], bf16, tag=f"wot{pair}", name=f"wot{pair}")
                        nc.sync.dma_start(out=t_, in_=Wo_d[pair])
                        WoT_sb[pair] = t_

            # ---- output projection ----
            for t in range(NPOS):
                for ch in range(DM // 512):
                    po = psp.tile([128, 512], f32, tag="ps", name="po")
                    for pair in range(8):
                        nc.tensor.matmul(po, oT_stack[pair][:, t * 128:(t + 1) * 128],
                                         WoT_sb[pair][:, ch * 512:(ch + 1) * 512],
                                         start=(pair == 0), stop=(pair == 7))
                    ot = outp.tile([128, 512], f32, tag="osb")
                    nc.vector.tensor_add(ot, po, bo_sb[:, ch * 512:(ch + 1) * 512])
                    nc.gpsimd.dma_start(out=out_d[t, :, ch * 512:(ch + 1) * 512], in_=ot)

    nc.compile()
    return nc


_PROG_CACHE = {}


def _get_program(causal: bool):
    key = bool(causal)
    if key not in _PROG_CACHE:
        if causal:
            blocks_per_pos = [2 * i + 2 for i in range(NPOS)]
            masked = {}
            for i in range(NPOS):
                masked[(i, 2 * i)] = 2 * i
                masked[(i, 2 * i + 1)] = 2 * i + 1
            nmask = 2 * NPOS
        else:
            blocks_per_pos = [NT] * NPOS
            masked = {(i, j): i * NT + j for i in range(NPOS) for j in range(NT)}
            nmask = NPOS * NT
        _PROG_CACHE[key] = (_build_program(blocks_per_pos, masked, nmask), masked, nmask)
    return _PROG_CACHE[key]


def _prep_inputs(q, k, v, Wq, bq, Wk, bk, Wv, bv, Wo, bo, mask, masked, nmask):
    A = (np.einsum('hde,hfe->hdf', Wk, Wq) * SCALE).astype(np.float32)   # [H,128,128]
    u = (np.einsum('hde,he->hd', Wk, bq) * SCALE).astype(np.float32)     # [H,128]
    WoT_host = np.ascontiguousarray(Wo.T.reshape(8, 128, DM)).astype(BF)
    bo_host = np.ascontiguousarray(bo[None, :]).astype(np.float32)
    mvalid = (mask[0, 0] != 0)                                           # [S(q), S(k)]

    in_maps = []
    tiles_by_core = []
    for c in range(NCORES):
        b, parity = c // 2, c % 2
        tiles = _core_tiles(parity)
        tiles_by_core.append(tiles)
        rows = np.concatenate([np.arange(t * 128, (t + 1) * 128) for t in tiles])
        qT = np.ascontiguousarray(q[b][:, rows, :].transpose(0, 2, 1)).astype(BF)
        # T2T[h] = (k[b,h] @ A_h)^T
        T2T = np.einsum('hsd,hdf->hfs', k[b], A).astype(BF)              # [H,128,S]
        # Vt[h, k_local, j, :] = [V[h, j*128+k_local, :], 1] * exp(w[h, j*128+k_local])
        # (the per-key exp bias is folded into Vt; softmax normalization
        # divides it back out exactly where it matters)
        V = (np.einsum('hsd,hde->hse', v[b], Wv) + bv[:, None, :]).astype(np.float32)
        wbv = np.exp(np.einsum('hsd,hd->hs', k[b], u)).astype(np.float32)  # [H,S]
        Vt = np.concatenate([V.reshape(H, NT, 128, PHD).transpose(0, 2, 1, 3),
                             np.ones((H, 128, NT, 1), np.float32)], axis=3)
        Vt *= wbv.reshape(H, NT, 128).transpose(0, 2, 1)[:, :, :, None]
        Vt = np.ascontiguousarray(Vt.reshape(H, 128, NT * 65)).astype(BF)
        mk_host = np.zeros((128, max(nmask, 1) * 128), np.float32)
        for (i, j), slot in masked.items():
            t = tiles[i]
            sub = mvalid[t * 128:(t + 1) * 128, j * 128:(j + 1) * 128]   # [q,k]
            mk_host[:, slot * 128:(slot + 1) * 128] = sub.T.astype(np.float32)
        in_maps.append({
            "qT": qT, "T2T": T2T, "Vt": Vt, "mk": mk_host.astype(BF),
            "WoT": WoT_host, "bo": bo_host,
        })
    return in_maps, tiles_by_core


def _is_causal(mask):
    m = np.asarray(mask[0, 0])
    expect = np.tri(S, S, dtype=np.int64)
    return bool(np.array_equal((m != 0), (expect != 0)))


def kernel(q, k, v, Wq, bq, Wk, bk, Wv, bv, Wo, bo, mask):
    q, k, v = (np.asarray(x, np.float32) for x in (q, k, v))
    Wq, bq, Wk, bk = (np.asarray(x, np.float32) for x in (Wq, bq, Wk, bk))
    Wv, bv, Wo, bo = (np.asarray(x, np.float32) for x in (Wv, bv, Wo, bo))
    mask = np.asarray(mask)

    causal = _is_causal(mask)
    nc, masked, nmask = _get_program(causal)
    in_maps, tiles_by_core = _prep_inputs(q, k, v, Wq, bq, Wk, bk, Wv, bv, Wo, bo,
                                          mask, masked, nmask)
    res = run_bass_kernel_spmd(nc, in_maps, core_ids=list(range(NCORES)))
    out_full = np.empty((B, S, DM), np.float32)
    for c in range(NCORES):
        b = c // 2
        oc = res.results[c]["out"]                                       # [NPOS,128,DM]
        for i, t in enumerate(tiles_by_core[c]):
            out_full[b, t * 128:(t + 1) * 128, :] = oc[i]
    return out_full
